# revision 39
# baseline (speedup 1.0000x reference)
"""Multi-head attention (with p_attn output) on 8 TRN2 NeuronCores.

Problem: B=2, H=16, S=2048, D=64 fp32 attention returning
(output[B,H,S,D], p_attn[B,H,S,S]).

Sharding: batch*head parallel — 32 heads split 4-per-core across 8 cores;
no cross-core communication.

Per-core kernel layout trick: everything is computed transposed
(S^T[k,q] = K @ Q^T) so that the PV contraction (over k) needs no
on-chip transposes:
  - S^T tile [k=128, q=512] = matmul(lhsT=K^T[d,kslice], rhs=Q^T[d,qslice]);
    K=64 contractions are row-packed two-at-a-time into array halves
    (partitions 0-63 / 64-127) so pairs run concurrently and LDWEIGHTS
    overlaps. Matmul inputs are bf16 (single-pass PE; fp32 runs a 2-pass
    LOW_HIGH decomposition at half speed), accumulation stays fp32.
  - E^T = exp(S^T / 8) on ScalarE, stored bf16.
  - matmul(lhsT=[1|V][k,65], rhs=E^T[k,q]) accumulated over k gives
    rowsum[q] (row 0) and PV^T[d,q] (rows 1..64) in fp32; contraction is
    split into partition halves for the same row-packing concurrency.
  - 1/rowsum (fp32 DVE reciprocal) -> bf16 -> broadcast to 128
    partitions on GpSimd.
  - p_attn^T = E^T * bcast on VectorE (all-bf16 2x mode, in-place),
    stored to fp32 DRAM via a casting SWDGE DMA.
Host side only does layout transforms (transpose/reshape/dup/cast), no
other math.
"""

import sys
import numpy as np
import ml_dtypes

if "/opt/trn_rl_repo" not in sys.path:
    sys.path.append("/opt/trn_rl_repo")

from concourse import bacc, tile, mybir
from concourse.bass_utils import run_bass_kernel_spmd

B, H, S, D = 2, 16, 2048, 64
N_CORES = 8
HPC = (B * H) // N_CORES  # heads per core = 4
KT = 16                   # k tiles of 128
KT2 = 8                   # pairs of k tiles
QC = 4                    # q chunks per head
QCS = 512                 # q chunk size

_CACHE = {}
LAST_RESULT = None


def _build():
    f32 = mybir.dt.float32
    bf16 = mybir.dt.bfloat16
    nc = bacc.Bacc("TRN2", target_bir_lowering=False, debug=False,
                   num_devices=N_CORES)

    # qt: Q^T duplicated into both partition halves [128, S]
    # kt: K^T k-tiles packed even->partitions 0:64, odd->64:128
    qt_d = nc.dram_tensor("qt", [HPC, 128, S], bf16, kind="ExternalInput")
    kt_d = nc.dram_tensor("kt", [HPC, 128, KT2, 128], bf16,
                          kind="ExternalInput")
    v_d = nc.dram_tensor("v", [HPC, 128, KT, D], bf16, kind="ExternalInput")
    pat_d = nc.dram_tensor("pat", [HPC, S, S], f32, kind="ExternalOutput")
    ot_d = nc.dram_tensor("ot", [HPC, D, S], f32, kind="ExternalOutput")

    with tile.TileContext(nc) as tc:
        with (
            tc.tile_pool(name="const", bufs=1) as const_pool,
            tc.tile_pool(name="qk", bufs=2) as qk_pool,
            tc.tile_pool(name="vp", bufs=2) as v_pool,
            tc.tile_pool(name="e", bufs=6) as e_pool,
            tc.tile_pool(name="small", bufs=4) as small_pool,
            tc.tile_pool(name="stp", bufs=2, space="PSUM") as st_psum,
            tc.tile_pool(name="pvp", bufs=2, space="PSUM") as pv_psum,
        ):
            zero_bias = const_pool.tile([128, 1], f32)
            nc.vector.memset(zero_bias[:], 0.0)

            head_tiles = {}

            def load_head(h):
                if h >= HPC or h in head_tiles:
                    return
                qt_s = qk_pool.tile([128, S], bf16, tag="qt")
                nc.scalar.dma_start(qt_s[:], qt_d.ap()[h])
                kt_s = qk_pool.tile([128, KT2, 128], bf16, tag="kt")
                nc.scalar.dma_start(kt_s[:], kt_d.ap()[h])
                # vone: col 0 = 1.0 (rowsum), cols 1..64 = V
                vone = v_pool.tile([128, KT, D + 1], bf16)
                nc.scalar.dma_start(vone[:, :, 1:D + 1], v_d.ap()[h])
                nc.vector.memset(vone[:, :, 0], 1.0)
                head_tiles[h] = (qt_s, kt_s, vone)

            def matmul_phase(h, q0, qn):
                qt_s, kt_s, vone = head_tiles[h]
                qsl = slice(q0, q0 + qn)
                e_t = e_pool.tile([128, KT, qn], bf16, tag="e_t")
                # S^T: pairs of K=64 matmuls in opposite array halves
                for k2 in range(KT2):
                    st = st_psum.tile([128, 2 * QCS], f32, tag="st")
                    nc.tensor.matmul(
                        st[:, 0:qn], kt_s[0:64, k2, :], qt_s[0:64, qsl],
                        start=True, stop=True)
                    # second half anchored at the QCS bank boundary so the
                    # concurrent row-tile matmuls never share a PSUM bank
                    nc.tensor.matmul(
                        st[:, QCS:QCS + qn], kt_s[64:128, k2, :],
                        qt_s[64:128, qsl], start=True, stop=True)
                    nc.scalar.activation(
                        e_t[:, 2 * k2:2 * k2 + 2, :],
                        st[:].rearrange("p (a b) -> p a b", a=2)[:, :, 0:qn],
                        mybir.ActivationFunctionType.Exp,
                        bias=zero_bias[:],
                        scale=0.125,
                    )
                # PV^T + rowsum: contraction split into partition halves
                pv = pv_psum.tile([D + 1, 2 * QCS], f32, tag="pv")
                for kti in range(KT):
                    nc.tensor.matmul(
                        pv[:, 0:qn], vone[0:64, kti, :],
                        e_t[0:64, kti, :],
                        start=(kti == 0), stop=(kti == KT - 1))
                    nc.tensor.matmul(
                        pv[:, QCS:QCS + qn], vone[64:128, kti, :],
                        e_t[64:128, kti, :],
                        start=(kti == 0), stop=(kti == KT - 1))
                return e_t, pv

            def extract(pv, qn):
                # combine halves + reciprocal of rowsum (frees pv)
                pvb = small_pool.tile([D + 1, QCS], f32, tag="pvb")
                nc.scalar.copy(pvb[:, 0:qn], pv[:, QCS:QCS + qn])
                pvs = small_pool.tile([D + 1, QCS], f32, tag="pvs")
                nc.vector.tensor_tensor(
                    pvs[:, 0:qn], pv[:, 0:qn], pvb[:, 0:qn],
                    mybir.AluOpType.add)
                recip = small_pool.tile([1, QCS], f32, tag="recip")
                nc.vector.reciprocal(recip[:, 0:qn], pvs[0:1, 0:qn])
                recipb = small_pool.tile([1, QCS], bf16, tag="recipb")
                nc.vector.tensor_copy(recipb[:, 0:qn], recip[:, 0:qn])
                # broadcast 1/rowsum to all 128 partitions on the idle
                # GpSimd engine (keeps PE mode-switch free)
                bcs = small_pool.tile([128, QCS], bf16, tag="bcs")
                nc.gpsimd.partition_broadcast(bcs[:, 0:qn], recipb[:, 0:qn])
                return pvs, bcs

            def finalize(h, q0, qn, e_t, pvs, bcs):
                # p_attn^T = E^T * (1/rowsum), broadcast along partitions.
                # Done in two k-halves so the (casting) store DMA starts
                # earlier and the DMA stream stays smooth.
                KH = KT // 2
                for half in range(2):
                    ksl = slice(half * KH, (half + 1) * KH)
                    nc.vector.tensor_tensor(
                        e_t[:, ksl, :], e_t[:, ksl, :],
                        bcs[:, None, 0:qn].broadcast_to([128, KH, qn]),
                        mybir.AluOpType.mult,
                    )
                    # bf16 -> f32 cast during the store (SWDGE only
                    # supports the cast, so both halves ride gpsimd, but
                    # issue them as separate DMAs for pipelining)
                    nc.gpsimd.dma_start(
                        pat_d.ap()[h]
                        .rearrange("(t p) q -> p t q", p=128)
                        [:, ksl, q0:q0 + qn],
                        e_t[:, ksl, :],
                    )
                # out^T = PV^T * (1/rowsum); row 0 of pvs is the rowsum
                # (multiplied to ~1.0 here, unused) so the TT partition
                # base stays 0.
                otn = small_pool.tile([D + 1, QCS], f32, tag="otn")
                nc.vector.tensor_tensor(
                    otn[:, 0:qn], pvs[:, 0:qn], bcs[0:D + 1, 0:qn],
                    mybir.AluOpType.mult,
                )
                nc.sync.dma_start(
                    ot_d.ap()[h][:, q0:q0 + qn],
                    otn[1:D + 1, 0:qn])

            # q-chunk schedule: 512-wide jobs, except the first and last
            # jobs are split into 256-wide micro-jobs to shorten the
            # pipeline ramp and drain.
            import os as _os
            micro = _os.environ.get("KERNEL_MICRO") == "1"
            jobs = []
            for h in range(HPC):
                chunks = [QCS] * QC
                if micro and h == 0:
                    chunks = [QCS // 2, QCS // 2] + [QCS] * (QC - 1)
                if micro and h == HPC - 1:
                    chunks = chunks[:-1] + [QCS // 2, QCS // 2]
                q0 = 0
                for qn in chunks:
                    jobs.append((h, q0, qn))
                    q0 += qn
            load_head(0)
            pending = []
            for ji, (h, q0, qn) in enumerate(jobs):
                if q0 == 0:
                    load_head(h + 1)  # prefetch next head's inputs
                e_t, pv = matmul_phase(h, q0, qn)
                pvs, bcs = extract(pv, qn)
                # steady-state: finalize two jobs behind (hides the
                # rowsum->reciprocal->broadcast chain)
                if len(pending) >= 2:
                    finalize(*pending.pop(0))
                pending.append((h, q0, qn, e_t, pvs, bcs))
                if q0 + qn == S and (h - 1) in head_tiles:
                    del head_tiles[h - 1]
            for p in pending:
                finalize(*p)

    nc.compile()
    return nc


def _get_nc():
    if "nc" not in _CACHE:
        _CACHE["nc"] = _build()
    return _CACHE["nc"]


def kernel(query, key, value):
    global LAST_RESULT
    q = np.asarray(query, dtype=np.float32)
    k = np.asarray(key, dtype=np.float32)
    v = np.asarray(value, dtype=np.float32)
    bf = ml_dtypes.bfloat16

    # Host-side layout prep (no math): Q^T (duplicated into both
    # partition halves), K^T (even/odd k-tile pack), V k-tiled; bf16.
    qt = q.reshape(B * H, S, D).transpose(0, 2, 1)
    qt2 = np.ascontiguousarray(
        np.concatenate([qt, qt], axis=1)).astype(bf)
    kt = k.reshape(B * H, S, D).transpose(0, 2, 1)  # [32, 64, 2048]
    kt2 = np.ascontiguousarray(
        kt.reshape(B * H, D, KT2, 2, 128)
        .transpose(0, 3, 1, 2, 4)
        .reshape(B * H, 128, KT2, 128)).astype(bf)
    vp = np.ascontiguousarray(
        v.reshape(B * H, KT, 128, D).transpose(0, 2, 1, 3)).astype(bf)

    in_maps = [
        {
            "qt": qt2[c * HPC:(c + 1) * HPC],
            "kt": kt2[c * HPC:(c + 1) * HPC],
            "v": vp[c * HPC:(c + 1) * HPC],
        }
        for c in range(N_CORES)
    ]

    nc = _get_nc()
    res = run_bass_kernel_spmd(nc, in_maps, core_ids=list(range(N_CORES)))
    LAST_RESULT = res

    pat = np.concatenate([r["pat"] for r in res.results])  # [32, S(k), S(q)]
    ot = np.concatenate([r["ot"] for r in res.results])    # [32, D, S(q)]

    p_attn = pat.reshape(B, H, S, S).swapaxes(2, 3)  # -> [B,H,q,k] view
    output = ot.reshape(B, H, D, S).swapaxes(2, 3)   # -> [B,H,q,d] view
    return output, p_attn


# revision 40
# speedup vs baseline: 1.1545x; 1.1545x over previous
"""Multi-head attention (with p_attn output) on 8 TRN2 NeuronCores.

Problem: B=2, H=16, S=2048, D=64 fp32 attention returning
(output[B,H,S,D], p_attn[B,H,S,S]).

Sharding: batch*head parallel — 32 heads split 4-per-core across 8 cores;
no cross-core communication.

Per-core kernel layout trick: everything is computed transposed
(S^T[k,q] = K @ Q^T) so that the PV contraction (over k) needs no
on-chip transposes:
  - S^T tile [k=128, q=512] = matmul(lhsT=K^T[d,kslice], rhs=Q^T[d,qslice]);
    K=64 contractions are row-packed two-at-a-time into array halves
    (partitions 0-63 / 64-127) so pairs run concurrently and LDWEIGHTS
    overlaps. Matmul inputs are bf16 (single-pass PE; fp32 runs a 2-pass
    LOW_HIGH decomposition at half speed), accumulation stays fp32.
  - E^T = exp(S^T / 8) on ScalarE, stored bf16.
  - matmul(lhsT=[1|V][k,65], rhs=E^T[k,q]) accumulated over k gives
    rowsum[q] (row 0) and PV^T[d,q] (rows 1..64) in fp32; contraction is
    split into partition halves for the same row-packing concurrency.
  - 1/rowsum (fp32 DVE reciprocal) -> bf16 -> broadcast to 128
    partitions on GpSimd.
  - p_attn^T = E^T * bcast on VectorE (all-bf16 2x mode, in-place),
    stored to fp32 DRAM via a casting SWDGE DMA.
Host side only does layout transforms (transpose/reshape/dup/cast), no
other math.
"""

import os
import sys
import types
import numpy as np
import ml_dtypes

if "/opt/trn_rl_repo" not in sys.path:
    sys.path.append("/opt/trn_rl_repo")

# If NTFF tracing is requested (BASS_TRACE=1), run_bass_kernel_spmd
# imports antenv.axon_hooks, which this image's antenv stub lacks.
# Provide it (backed by the libaxon profiling entry points) so tracing
# works instead of crashing; a no-op when tracing is off.
if os.environ.get("BASS_TRACE"):
    try:
        import antenv.axon_hooks  # noqa: F401
    except ImportError:
        try:
            from trn_agent_boot.trn_boot import _ntff_profile_via_ctypes
            _hook = _ntff_profile_via_ctypes("/opt/axon/libaxon_pjrt.so")
        except Exception:
            _hook = None
        _mod = types.ModuleType("antenv.axon_hooks")
        _mod.get_axon_ntff_profile_hook = lambda: _hook
        sys.modules["antenv.axon_hooks"] = _mod

from concourse import bacc, tile, mybir
from concourse.bass_utils import run_bass_kernel_spmd

B, H, S, D = 2, 16, 2048, 64
N_CORES = 8
HPC = (B * H) // N_CORES  # heads per core = 4
KT = 16                   # k tiles of 128
KT2 = 8                   # pairs of k tiles
QC = 4                    # q chunks per head
QCS = 512                 # q chunk size

_CACHE = {}
LAST_RESULT = None


def _build():
    f32 = mybir.dt.float32
    bf16 = mybir.dt.bfloat16
    nc = bacc.Bacc("TRN2", target_bir_lowering=False, debug=False,
                   num_devices=N_CORES)

    # qt: Q^T duplicated into both partition halves [128, S]
    # kt: K^T k-tiles packed even->partitions 0:64, odd->64:128
    qt_d = nc.dram_tensor("qt", [HPC, 128, S], bf16, kind="ExternalInput")
    kt_d = nc.dram_tensor("kt", [HPC, 128, KT2, 128], bf16,
                          kind="ExternalInput")
    v_d = nc.dram_tensor("v", [HPC, 128, KT, D], bf16, kind="ExternalInput")
    pat_d = nc.dram_tensor("pat", [HPC, S, S], f32, kind="ExternalOutput")
    ot_d = nc.dram_tensor("ot", [HPC, D, S], f32, kind="ExternalOutput")

    with tile.TileContext(nc) as tc:
        with (
            tc.tile_pool(name="const", bufs=1) as const_pool,
            tc.tile_pool(name="qk", bufs=2) as qk_pool,
            tc.tile_pool(name="vp", bufs=2) as v_pool,
            tc.tile_pool(name="e", bufs=5) as e_pool,
            tc.tile_pool(name="small", bufs=4) as small_pool,
            tc.tile_pool(name="stp", bufs=2, space="PSUM") as st_psum,
            tc.tile_pool(name="pvp", bufs=2, space="PSUM") as pv_psum,
        ):
            zero_bias = const_pool.tile([128, 1], f32)
            nc.vector.memset(zero_bias[:], 0.0)

            head_tiles = {}

            def load_head(h):
                if h >= HPC or h in head_tiles:
                    return
                qt_s = qk_pool.tile([128, S], bf16, tag="qt")
                nc.scalar.dma_start(qt_s[:], qt_d.ap()[h])
                kt_s = qk_pool.tile([128, KT2, 128], bf16, tag="kt")
                nc.scalar.dma_start(kt_s[:], kt_d.ap()[h])
                # vone: col 0 = 1.0 (rowsum), cols 1..64 = V
                vone = v_pool.tile([128, KT, D + 1], bf16)
                nc.scalar.dma_start(vone[:, :, 1:D + 1], v_d.ap()[h])
                nc.vector.memset(vone[:, :, 0], 1.0)
                head_tiles[h] = (qt_s, kt_s, vone)

            def matmul_phase(h, q0, qn):
                qt_s, kt_s, vone = head_tiles[h]
                qsl = slice(q0, q0 + qn)
                e_t = e_pool.tile([128, KT, qn], bf16, tag="e_t")
                # S^T: pairs of K=64 matmuls in opposite array halves
                for k2 in range(KT2):
                    st = st_psum.tile([128, 2 * QCS], f32, tag="st")
                    nc.tensor.matmul(
                        st[:, 0:qn], kt_s[0:64, k2, :], qt_s[0:64, qsl],
                        start=True, stop=True)
                    # second half anchored at the QCS bank boundary so the
                    # concurrent row-tile matmuls never share a PSUM bank
                    nc.tensor.matmul(
                        st[:, QCS:QCS + qn], kt_s[64:128, k2, :],
                        qt_s[64:128, qsl], start=True, stop=True)
                    nc.scalar.activation(
                        e_t[:, 2 * k2:2 * k2 + 2, :],
                        st[:].rearrange("p (a b) -> p a b", a=2)[:, :, 0:qn],
                        mybir.ActivationFunctionType.Exp,
                        bias=zero_bias[:],
                        scale=0.125,
                    )
                # PV^T + rowsum: contraction split into partition halves
                pv = pv_psum.tile([D + 1, 2 * QCS], f32, tag="pv")
                for kti in range(KT):
                    nc.tensor.matmul(
                        pv[:, 0:qn], vone[0:64, kti, :],
                        e_t[0:64, kti, :],
                        start=(kti == 0), stop=(kti == KT - 1))
                    nc.tensor.matmul(
                        pv[:, QCS:QCS + qn], vone[64:128, kti, :],
                        e_t[64:128, kti, :],
                        start=(kti == 0), stop=(kti == KT - 1))
                return e_t, pv

            def extract(pv, qn):
                # combine halves + reciprocal of rowsum (frees pv)
                pvb = small_pool.tile([D + 1, QCS], f32, tag="pvb")
                nc.scalar.copy(pvb[:, 0:qn], pv[:, QCS:QCS + qn])
                pvs = small_pool.tile([D + 1, QCS], f32, tag="pvs")
                nc.vector.tensor_tensor(
                    pvs[:, 0:qn], pv[:, 0:qn], pvb[:, 0:qn],
                    mybir.AluOpType.add)
                recip = small_pool.tile([1, QCS], f32, tag="recip")
                nc.vector.reciprocal(recip[:, 0:qn], pvs[0:1, 0:qn])
                recipb = small_pool.tile([1, QCS], bf16, tag="recipb")
                nc.vector.tensor_copy(recipb[:, 0:qn], recip[:, 0:qn])
                # broadcast 1/rowsum to all 128 partitions on the idle
                # GpSimd engine (keeps PE mode-switch free)
                bcs = small_pool.tile([128, QCS], bf16, tag="bcs")
                nc.gpsimd.partition_broadcast(bcs[:, 0:qn], recipb[:, 0:qn])
                return pvs, bcs

            def finalize(h, q0, qn, e_t, pvs, bcs):
                # p_attn^T = E^T * (1/rowsum), broadcast along partitions.
                # Done in two k-halves so the (casting) store DMA starts
                # earlier and the DMA stream stays smooth.
                KH = KT // 2
                for half in range(2):
                    ksl = slice(half * KH, (half + 1) * KH)
                    nc.vector.tensor_tensor(
                        e_t[:, ksl, :], e_t[:, ksl, :],
                        bcs[:, None, 0:qn].broadcast_to([128, KH, qn]),
                        mybir.AluOpType.mult,
                    )
                    # bf16 -> f32 cast during the store (SWDGE only
                    # supports the cast, so both halves ride gpsimd, but
                    # issue them as separate DMAs for pipelining)
                    nc.gpsimd.dma_start(
                        pat_d.ap()[h]
                        .rearrange("(t p) q -> p t q", p=128)
                        [:, ksl, q0:q0 + qn],
                        e_t[:, ksl, :],
                    )
                # out^T = PV^T * (1/rowsum); row 0 of pvs is the rowsum
                # (multiplied to ~1.0 here, unused) so the TT partition
                # base stays 0.
                otn = small_pool.tile([D + 1, QCS], f32, tag="otn")
                nc.vector.tensor_tensor(
                    otn[:, 0:qn], pvs[:, 0:qn], bcs[0:D + 1, 0:qn],
                    mybir.AluOpType.mult,
                )
                nc.sync.dma_start(
                    ot_d.ap()[h][:, q0:q0 + qn],
                    otn[1:D + 1, 0:qn])

            # q-chunk schedule: 512-wide jobs, except the first and last
            # jobs are split into 256-wide micro-jobs to shorten the
            # pipeline ramp and drain.
            import os as _os
            micro = _os.environ.get("KERNEL_MICRO") == "1"
            jobs = []
            for h in range(HPC):
                chunks = [QCS] * QC
                if micro and h == 0:
                    chunks = [QCS // 2, QCS // 2] + [QCS] * (QC - 1)
                if micro and h == HPC - 1:
                    chunks = chunks[:-1] + [QCS // 2, QCS // 2]
                q0 = 0
                for qn in chunks:
                    jobs.append((h, q0, qn))
                    q0 += qn
            load_head(0)
            pending = []
            for ji, (h, q0, qn) in enumerate(jobs):
                if q0 == 0:
                    load_head(h + 1)  # prefetch next head's inputs
                e_t, pv = matmul_phase(h, q0, qn)
                pvs, bcs = extract(pv, qn)
                # steady-state: finalize two jobs behind (hides the
                # rowsum->reciprocal->broadcast chain)
                if len(pending) >= 2:
                    finalize(*pending.pop(0))
                pending.append((h, q0, qn, e_t, pvs, bcs))
                if q0 + qn == S and (h - 1) in head_tiles:
                    del head_tiles[h - 1]
            for p in pending:
                finalize(*p)

    nc.compile()
    return nc


def _get_nc():
    if "nc" not in _CACHE:
        _CACHE["nc"] = _build()
    return _CACHE["nc"]


def kernel(query, key, value):
    global LAST_RESULT
    q = np.asarray(query, dtype=np.float32)
    k = np.asarray(key, dtype=np.float32)
    v = np.asarray(value, dtype=np.float32)
    bf = ml_dtypes.bfloat16

    # Host-side layout prep (no math): Q^T (duplicated into both
    # partition halves), K^T (even/odd k-tile pack), V k-tiled; bf16.
    qt = q.reshape(B * H, S, D).transpose(0, 2, 1)
    qt2 = np.ascontiguousarray(
        np.concatenate([qt, qt], axis=1)).astype(bf)
    kt = k.reshape(B * H, S, D).transpose(0, 2, 1)  # [32, 64, 2048]
    kt2 = np.ascontiguousarray(
        kt.reshape(B * H, D, KT2, 2, 128)
        .transpose(0, 3, 1, 2, 4)
        .reshape(B * H, 128, KT2, 128)).astype(bf)
    vp = np.ascontiguousarray(
        v.reshape(B * H, KT, 128, D).transpose(0, 2, 1, 3)).astype(bf)

    in_maps = [
        {
            "qt": qt2[c * HPC:(c + 1) * HPC],
            "kt": kt2[c * HPC:(c + 1) * HPC],
            "v": vp[c * HPC:(c + 1) * HPC],
        }
        for c in range(N_CORES)
    ]

    nc = _get_nc()
    res = run_bass_kernel_spmd(nc, in_maps, core_ids=list(range(N_CORES)))
    LAST_RESULT = res

    pat = np.concatenate([r["pat"] for r in res.results])  # [32, S(k), S(q)]
    ot = np.concatenate([r["ot"] for r in res.results])    # [32, D, S(q)]

    p_attn = pat.reshape(B, H, S, S).swapaxes(2, 3)  # -> [B,H,q,k] view
    output = ot.reshape(B, H, D, S).swapaxes(2, 3)   # -> [B,H,q,d] view
    return output, p_attn


# revision 41
# speedup vs baseline: 1.1597x; 1.0046x over previous
"""Multi-head attention (with p_attn output) on 8 TRN2 NeuronCores.

Problem: B=2, H=16, S=2048, D=64 fp32 attention returning
(output[B,H,S,D], p_attn[B,H,S,S]).

Sharding: batch*head parallel — 32 heads split 4-per-core across 8 cores;
no cross-core communication.

Per-core kernel layout trick: everything is computed transposed
(S^T[k,q] = K @ Q^T) so that the PV contraction (over k) needs no
on-chip transposes:
  - S^T tile [k=128, q=512] = matmul(lhsT=K^T[d,kslice], rhs=Q^T[d,qslice]);
    K=64 contractions are row-packed two-at-a-time into array halves
    (partitions 0-63 / 64-127) so pairs run concurrently and LDWEIGHTS
    overlaps. Matmul inputs are bf16 (single-pass PE; fp32 runs a 2-pass
    LOW_HIGH decomposition at half speed), accumulation stays fp32.
  - E^T = exp(S^T / 8) on ScalarE, stored bf16.
  - matmul(lhsT=[1|V][k,65], rhs=E^T[k,q]) accumulated over k gives
    rowsum[q] (row 0) and PV^T[d,q] (rows 1..64) in fp32; contraction is
    split into partition halves for the same row-packing concurrency.
  - 1/rowsum (fp32 DVE reciprocal) -> bf16 -> broadcast to 128
    partitions on GpSimd.
  - p_attn^T = E^T * bcast on VectorE (all-bf16 2x mode, in-place),
    stored to fp32 DRAM via a casting SWDGE DMA.
Host side only does layout transforms (transpose/reshape/dup/cast), no
other math.
"""

import os
import sys
import types
import numpy as np
import ml_dtypes

if "/opt/trn_rl_repo" not in sys.path:
    sys.path.append("/opt/trn_rl_repo")

# If NTFF tracing is requested (BASS_TRACE=1), run_bass_kernel_spmd
# imports antenv.axon_hooks, which this image's antenv stub lacks.
# Provide it (backed by the libaxon profiling entry points) so tracing
# works instead of crashing; a no-op when tracing is off.
if os.environ.get("BASS_TRACE"):
    try:
        import antenv.axon_hooks  # noqa: F401
    except ImportError:
        try:
            from trn_agent_boot.trn_boot import _ntff_profile_via_ctypes
            _hook = _ntff_profile_via_ctypes("/opt/axon/libaxon_pjrt.so")
        except Exception:
            _hook = None
        _mod = types.ModuleType("antenv.axon_hooks")
        _mod.get_axon_ntff_profile_hook = lambda: _hook
        sys.modules["antenv.axon_hooks"] = _mod

from concourse import bacc, tile, mybir
from concourse.bass_utils import run_bass_kernel_spmd

B, H, S, D = 2, 16, 2048, 64
N_CORES = 8
HPC = (B * H) // N_CORES  # heads per core = 4
KT = 16                   # k tiles of 128
KT2 = 8                   # pairs of k tiles
QC = 4                    # q chunks per head
QCS = 512                 # q chunk size

_CACHE = {}
LAST_RESULT = None


def _build():
    f32 = mybir.dt.float32
    bf16 = mybir.dt.bfloat16
    nc = bacc.Bacc("TRN2", target_bir_lowering=False, debug=False,
                   num_devices=N_CORES)

    # qt: Q^T duplicated into both partition halves [128, S]
    # kt: K^T k-tiles packed even->partitions 0:64, odd->64:128
    qt_d = nc.dram_tensor("qt", [HPC, 128, S], bf16, kind="ExternalInput")
    kt_d = nc.dram_tensor("kt", [HPC, 128, KT2, 128], bf16,
                          kind="ExternalInput")
    v_d = nc.dram_tensor("v", [HPC, 128, KT, D], bf16, kind="ExternalInput")
    pat_d = nc.dram_tensor("pat", [HPC, S, S], f32, kind="ExternalOutput")
    ot_d = nc.dram_tensor("ot", [HPC, D, S], f32, kind="ExternalOutput")

    with tile.TileContext(nc) as tc:
        with (
            tc.tile_pool(name="const", bufs=1) as const_pool,
            tc.tile_pool(name="qk", bufs=2) as qk_pool,
            tc.tile_pool(name="vp", bufs=2) as v_pool,
            tc.tile_pool(name="e", bufs=5) as e_pool,
            tc.tile_pool(name="small", bufs=4) as small_pool,
            tc.tile_pool(name="stp", bufs=2, space="PSUM") as st_psum,
            tc.tile_pool(name="pvp", bufs=2, space="PSUM") as pv_psum,
        ):
            zero_bias = const_pool.tile([128, 1], f32)
            nc.vector.memset(zero_bias[:], 0.0)

            head_tiles = {}

            def load_head(h):
                if h >= HPC or h in head_tiles:
                    return
                qt_s = qk_pool.tile([128, S], bf16, tag="qt")
                nc.scalar.dma_start(qt_s[:], qt_d.ap()[h])
                kt_s = qk_pool.tile([128, KT2, 128], bf16, tag="kt")
                nc.scalar.dma_start(kt_s[:], kt_d.ap()[h])
                # vone: col 0 = 1.0 (rowsum), cols 1..64 = V
                vone = v_pool.tile([128, KT, D + 1], bf16)
                nc.scalar.dma_start(vone[:, :, 1:D + 1], v_d.ap()[h])
                nc.vector.memset(vone[:, :, 0], 1.0)
                head_tiles[h] = (qt_s, kt_s, vone)

            def matmul_phase(h, q0, qn):
                qt_s, kt_s, vone = head_tiles[h]
                qsl = slice(q0, q0 + qn)
                e_t = e_pool.tile([128, KT, qn], bf16, tag="e_t")
                # S^T: pairs of K=64 matmuls in opposite array halves
                for k2 in range(KT2):
                    st = st_psum.tile([128, 2 * QCS], f32, tag="st")
                    nc.tensor.matmul(
                        st[:, 0:qn], kt_s[0:64, k2, :], qt_s[0:64, qsl],
                        start=True, stop=True)
                    # second half anchored at the QCS bank boundary so the
                    # concurrent row-tile matmuls never share a PSUM bank
                    nc.tensor.matmul(
                        st[:, QCS:QCS + qn], kt_s[64:128, k2, :],
                        qt_s[64:128, qsl], start=True, stop=True)
                    nc.scalar.activation(
                        e_t[:, 2 * k2:2 * k2 + 2, :],
                        st[:].rearrange("p (a b) -> p a b", a=2)[:, :, 0:qn],
                        mybir.ActivationFunctionType.Exp,
                        bias=zero_bias[:],
                        scale=0.125,
                    )
                # PV^T + rowsum: contraction split into partition halves
                pv = pv_psum.tile([D + 1, 2 * QCS], f32, tag="pv")
                for kti in range(KT):
                    nc.tensor.matmul(
                        pv[:, 0:qn], vone[0:64, kti, :],
                        e_t[0:64, kti, :],
                        start=(kti == 0), stop=(kti == KT - 1))
                    nc.tensor.matmul(
                        pv[:, QCS:QCS + qn], vone[64:128, kti, :],
                        e_t[64:128, kti, :],
                        start=(kti == 0), stop=(kti == KT - 1))
                return e_t, pv

            def extract(pv, qn):
                # combine halves + reciprocal of rowsum (frees pv)
                pvb = small_pool.tile([D + 1, QCS], f32, tag="pvb")
                nc.scalar.copy(pvb[:, 0:qn], pv[:, QCS:QCS + qn])
                pvs = small_pool.tile([D + 1, QCS], f32, tag="pvs")
                nc.vector.tensor_tensor(
                    pvs[:, 0:qn], pv[:, 0:qn], pvb[:, 0:qn],
                    mybir.AluOpType.add)
                recip = small_pool.tile([1, QCS], f32, tag="recip")
                nc.vector.reciprocal(recip[:, 0:qn], pvs[0:1, 0:qn])
                recipb = small_pool.tile([1, QCS], bf16, tag="recipb")
                nc.vector.tensor_copy(recipb[:, 0:qn], recip[:, 0:qn])
                # broadcast 1/rowsum to all 128 partitions on the idle
                # GpSimd engine (keeps PE mode-switch free)
                bcs = small_pool.tile([128, QCS], bf16, tag="bcs")
                nc.gpsimd.partition_broadcast(bcs[:, 0:qn], recipb[:, 0:qn])
                return pvs, bcs

            def finalize(h, q0, qn, e_t, pvs, bcs):
                # p_attn^T = E^T * (1/rowsum), broadcast along partitions.
                # Done in two k-halves so the (casting) store DMA starts
                # earlier and the DMA stream stays smooth.
                KH = KT // 2
                for half in range(2):
                    ksl = slice(half * KH, (half + 1) * KH)
                    nc.vector.tensor_tensor(
                        e_t[:, ksl, :], e_t[:, ksl, :],
                        bcs[:, None, 0:qn].broadcast_to([128, KH, qn]),
                        mybir.AluOpType.mult,
                    )
                    # bf16 -> f32 cast during the store (SWDGE only
                    # supports the cast, so both halves ride gpsimd, but
                    # issue them as separate DMAs for pipelining)
                    nc.gpsimd.dma_start(
                        pat_d.ap()[h]
                        .rearrange("(t p) q -> p t q", p=128)
                        [:, ksl, q0:q0 + qn],
                        e_t[:, ksl, :],
                    )
                # out^T = PV^T * (1/rowsum); row 0 of pvs is the rowsum
                # (multiplied to ~1.0 here, unused) so the TT partition
                # base stays 0.
                otn = small_pool.tile([D + 1, QCS], f32, tag="otn")
                nc.vector.tensor_tensor(
                    otn[:, 0:qn], pvs[:, 0:qn], bcs[0:D + 1, 0:qn],
                    mybir.AluOpType.mult,
                )
                nc.sync.dma_start(
                    ot_d.ap()[h][:, q0:q0 + qn],
                    otn[1:D + 1, 0:qn])

            # q-chunk schedule: 512-wide jobs, except the first and last
            # jobs are split into 256-wide micro-jobs to shorten the
            # pipeline ramp and drain.
            micro = os.environ.get("KERNEL_MICRO") == "1"
            jobs = []
            for h in range(HPC):
                chunks = [QCS] * QC
                if micro and h == 0:
                    chunks = [QCS // 2, QCS // 2] + [QCS] * (QC - 1)
                if micro and h == HPC - 1:
                    chunks = chunks[:-1] + [QCS // 2, QCS // 2]
                q0 = 0
                for qn in chunks:
                    jobs.append((h, q0, qn))
                    q0 += qn
            load_head(0)
            pending = []
            for ji, (h, q0, qn) in enumerate(jobs):
                if q0 == 0:
                    load_head(h + 1)  # prefetch next head's inputs
                e_t, pv = matmul_phase(h, q0, qn)
                pvs, bcs = extract(pv, qn)
                # steady-state: finalize two jobs behind (hides the
                # rowsum->reciprocal->broadcast chain)
                if len(pending) >= 2:
                    finalize(*pending.pop(0))
                pending.append((h, q0, qn, e_t, pvs, bcs))
                if q0 + qn == S and (h - 1) in head_tiles:
                    del head_tiles[h - 1]
            for p in pending:
                finalize(*p)

    nc.compile()
    return nc


def _get_nc():
    if "nc" not in _CACHE:
        _CACHE["nc"] = _build()
    return _CACHE["nc"]


def kernel(query, key, value):
    global LAST_RESULT
    q = np.asarray(query, dtype=np.float32)
    k = np.asarray(key, dtype=np.float32)
    v = np.asarray(value, dtype=np.float32)
    bf = ml_dtypes.bfloat16

    # Host-side layout prep (no math): Q^T (duplicated into both
    # partition halves), K^T (even/odd k-tile pack), V k-tiled; bf16.
    qt = q.reshape(B * H, S, D).transpose(0, 2, 1)
    qt2 = np.ascontiguousarray(
        np.concatenate([qt, qt], axis=1)).astype(bf)
    kt = k.reshape(B * H, S, D).transpose(0, 2, 1)  # [32, 64, 2048]
    kt2 = np.ascontiguousarray(
        kt.reshape(B * H, D, KT2, 2, 128)
        .transpose(0, 3, 1, 2, 4)
        .reshape(B * H, 128, KT2, 128)).astype(bf)
    vp = np.ascontiguousarray(
        v.reshape(B * H, KT, 128, D).transpose(0, 2, 1, 3)).astype(bf)

    in_maps = [
        {
            "qt": qt2[c * HPC:(c + 1) * HPC],
            "kt": kt2[c * HPC:(c + 1) * HPC],
            "v": vp[c * HPC:(c + 1) * HPC],
        }
        for c in range(N_CORES)
    ]

    nc = _get_nc()
    res = run_bass_kernel_spmd(nc, in_maps, core_ids=list(range(N_CORES)))
    LAST_RESULT = res

    pat = np.concatenate([r["pat"] for r in res.results])  # [32, S(k), S(q)]
    ot = np.concatenate([r["ot"] for r in res.results])    # [32, D, S(q)]

    p_attn = pat.reshape(B, H, S, S).swapaxes(2, 3)  # -> [B,H,q,k] view
    output = ot.reshape(B, H, D, S).swapaxes(2, 3)   # -> [B,H,q,d] view
    return output, p_attn
